# revision 13
# baseline (speedup 1.0000x reference)
"""Trainium2 Bass kernel for the GRU caption model.

h0 = feat @ W_hp.T + b_hp; 200-step GRU whose hidden-proj gate pre-activations
are step-invariant; logits = h_t @ W_out.T + b_out -> [B, V, T].

v2 design (CoreSim cost-model driven):
- Vocab sharded 8 ways; every core runs the (latency-bound) GRU redundantly.
- GRU per-step chain minimized: step-invariant gate constants are preloaded
  into PSUM with a single N=384 fp32r identity matmul, the elementwise chain
  runs on Pool (GPSIMD, no SBUF/PSUM access-latency penalty) with the two
  tanhs on Act.  h_t is written once, in bf16, straight into the resT
  activation buffer that feeds both the next step's matmul and the
  projection.
- Projection in 4 t-blocks of 50 steps, bf16 weights/activations (1 cyc/row
  at any N), bias folded into the PSUM->SBUF copy (split between DVE+Pool).
- Output DMA: one transfer per (vocab-tile, t-block) with 6400-byte
  contiguous runs (full 360 GB/s); per-block DRAM tensors laid out
  [VPAD, B, BS], transposed/stitched on the host.
"""

import numpy as np
import ml_dtypes

import concourse.bass as bass
import concourse.mybir as mybir
import concourse.tile as tile
from concourse import bacc
from concourse.bass_utils import run_bass_kernel_spmd

F32 = mybir.dt.float32
F32R = mybir.dt.float32r
BF16 = mybir.dt.bfloat16
AF = mybir.ActivationFunctionType
ALU = mybir.AluOpType

VOCAB = 30522
HID = 512
FEAT = 2048
STEPS = 200
BATCH = 32
SOS = 101
NCORES = 8
P = 128
KO = HID // P          # 4 h-chunks
GM = 3 * HID // P      # 12 gate row-groups (r: 0-3, z: 4-7, n: 8-11)
KF = FEAT // P         # 16 feat chunks
VPAD = 3840            # per-core padded vocab rows = 30 * 128
MT = VPAD // P         # 30 vocab tiles per core
NBLK = 4
BS = STEPS // NBLK     # 50 steps per proj block
PBG = 4                # batch group size in proj (N = PBG*BS = 200)
UNITS_PER_STEP = 3     # proj (m,g) units interleaved per GRU step

LAST_RESULTS = None  # test harness introspection


def build():
    nc = bacc.Bacc("TRN2", target_bir_lowering=False, debug=False)

    featT = nc.dram_tensor("featT", [FEAT, BATCH], F32, kind="ExternalInput")
    WhpT = nc.dram_tensor("WhpT", [FEAT, HID], F32, kind="ExternalInput")
    WihT = nc.dram_tensor("WihT", [HID, 3 * HID], BF16, kind="ExternalInput")
    WhhT = nc.dram_tensor("WhhT", [HID, 3 * HID], F32, kind="ExternalInput")
    b_ih = nc.dram_tensor("b_ih", [3 * HID], F32, kind="ExternalInput")
    b_hh = nc.dram_tensor("b_hh", [3 * HID], F32, kind="ExternalInput")
    b_hp = nc.dram_tensor("b_hp", [HID], F32, kind="ExternalInput")
    x0T = nc.dram_tensor("x0T", [HID, BATCH], BF16, kind="ExternalInput")
    ident = nc.dram_tensor("ident", [P, P], F32, kind="ExternalInput")
    WoutT = nc.dram_tensor("WoutT", [HID, VPAD], BF16, kind="ExternalInput")
    b_out = nc.dram_tensor("b_out", [VPAD], F32, kind="ExternalInput")
    OUTS = [
        nc.dram_tensor(f"OUT{j}", [VPAD, BATCH, BS], F32, kind="ExternalOutput")
        for j in range(NBLK)
    ]

    with tile.TileContext(nc) as tc:
        with (
            tc.tile_pool(name="const", bufs=1) as const,
            tc.tile_pool(name="sp", bufs=4) as sp,
            tc.tile_pool(name="stg", bufs=3) as stg,
            tc.tile_pool(name="psg", bufs=2, space="PSUM") as psg,
            tc.tile_pool(name="psp", bufs=4, space="PSUM") as psp,
        ):
            # ---- constants into SBUF (few, large DMAs) ----
            featT_sb = const.tile([P, KF, BATCH], F32, tag="featsb")
            nc.sync.dma_start(featT_sb[:], featT.rearrange("(k p) b -> p k b", p=P))
            whp_sb = const.tile([P, KF, HID], F32, tag="whp")
            nc.sync.dma_start(whp_sb[:], WhpT.rearrange("(k p) h -> p k h", p=P))
            bhp_sb = const.tile([P, KO], F32, tag="bhp")
            nc.sync.dma_start(bhp_sb[:], b_hp.rearrange("(m p) -> p m", p=P))
            bih_sb = const.tile([P, GM], F32, tag="bih")
            nc.sync.dma_start(bih_sb[:], b_ih.rearrange("(m p) -> p m", p=P))
            bhh_sb = const.tile([P, GM], F32, tag="bhh")
            nc.sync.dma_start(bhh_sb[:], b_hh.rearrange("(m p) -> p m", p=P))
            id_sb = const.tile([P, P], F32, tag="id")
            nc.sync.dma_start(id_sb[:], ident[:, :])
            x0_sb = const.tile([P, KO, BATCH], BF16, tag="x0")
            nc.sync.dma_start(x0_sb[:], x0T.rearrange("(k p) b -> p k b", p=P))
            whh_sb = const.tile([P, KO, 3 * HID], F32, tag="whh")
            nc.sync.dma_start(whh_sb[:], WhhT.rearrange("(k p) g -> p k g", p=P))
            wih = const.tile([P, KO, GM, P], BF16, tag="wih")
            nc.sync.dma_start(
                wih[:], WihT.rearrange("(k p) (m c) -> p k m c", p=P, c=P)
            )
            wout_sb = const.tile([P, KO, VPAD], BF16, tag="wout")
            nc.sync.dma_start(
                wout_sb[:], WoutT.rearrange("(k p) v -> p k v", p=P)
            )
            bout_sb = const.tile([P, MT], F32, tag="bout")
            nc.sync.dma_start(bout_sb[:], b_out.rearrange("(m p) -> p m", p=P))

            # ---- h0 = feat @ W_hp.T + b_hp (fp32, exact) ----
            ps_h = psg.tile([P, GM, BATCH], F32, tag="gates", name="psh")
            for ko in range(KO):
                for kf in range(KF):
                    nc.tensor.matmul(
                        ps_h[:, ko, :],
                        whp_sb[:, kf, ko * P:(ko + 1) * P],
                        featT_sb[:, kf, :],
                        start=(kf == 0), stop=(kf == KF - 1),
                    )
            h0T = const.tile([P, KO, BATCH], F32, tag="h0T")
            for ko in range(KO):
                nc.scalar.activation(
                    h0T[:, ko, :], ps_h[:, ko, :], AF.Identity,
                    bias=bhp_sb[:, ko, None], scale=1.0,
                )
            h0_half = const.tile([P, KO, BATCH], F32, tag="h0h")
            nc.scalar.mul(h0_half[:], h0T[:], 0.5)

            # ---- gh = h0 @ W_hh.T + b_hh (fp32, exact; step-invariant) ----
            ps_g = psg.tile([P, GM, BATCH], F32, tag="gates", name="psg2")
            for m in range(GM):
                for k in range(KO):
                    nc.tensor.matmul(
                        ps_g[:, m, :],
                        whh_sb[:, k, m * P:(m + 1) * P],
                        h0T[:, k, :],
                        start=(k == 0), stop=(k == KO - 1),
                    )
            ghT = const.tile([P, GM, BATCH], F32, tag="ghT")
            for m in range(GM):
                nc.scalar.activation(
                    ghT[:, m, :], ps_g[:, m, :], AF.Identity,
                    bias=bhh_sb[:, m, None], scale=1.0,
                )
            # hn2 = 0.5*gh_n (used every step by the r-gating of n)
            hn2 = const.tile([P, KO, BATCH], F32, tag="hn2")
            nc.scalar.mul(hn2[:], ghT[:, 8:12, :], 0.5)
            # C: per-step PSUM preload. rz: gh_rz + b_ih_rz ; n: hn2 + b_ih_n
            C = const.tile([P, GM, BATCH], F32, tag="C")
            nc.vector.tensor_add(
                C[:, 0:8, :], ghT[:, 0:8, :],
                bih_sb[:, 0:8, None].to_broadcast((P, 8, BATCH)),
            )
            nc.vector.tensor_add(
                C[:, 8:12, :], hn2[:],
                bih_sb[:, 8:12, None].to_broadcast((P, KO, BATCH)),
            )

            # resT blocks: h_t in bf16, feeds next-step matmul AND projection
            resT = []
            for j in range(NBLK):
                rt = const.tile(
                    [P, KO, BATCH, BS], BF16, tag=f"resT{j}", name=f"resT{j}"
                )
                resT.append(rt)

            id_r = id_sb.bitcast(F32R)
            C_r = C.bitcast(F32R)

            NG = BATCH // PBG  # 8 batch groups per vocab tile
            proj_fifo = []     # pending (j, m, g) units
            stage_cur = [None]
            tail = [False]

            def emit_unit():
                j, m, g = proj_fifo.pop(0)
                if g == 0:
                    stage_cur[0] = stg.tile(
                        [P, BATCH, BS], F32, tag="stage", name="stage"
                    )
                stage = stage_cur[0]
                pp = psp.tile([P, PBG, BS], F32, tag="pp", name="pp")
                for k in range(KO):
                    nc.tensor.matmul(
                        pp[:, :, :],
                        wout_sb[:, k, m * P:(m + 1) * P],
                        resT[j][:, k, PBG * g:PBG * g + PBG, :],
                        start=(k == 0), stop=(k == KO - 1),
                    )
                dst = stage[:, PBG * g:PBG * g + PBG, :]
                # During the GRU, copies live on DVE only (Pool/Act host the
                # latency-critical chain); in the tail all engines pitch in.
                if tail[0] and g % 3 == 0:
                    nc.gpsimd.tensor_scalar_add(dst, pp[:, :, :], bout_sb[:, m, None])
                elif tail[0] and g % 3 == 1:
                    nc.scalar.activation(
                        dst, pp[:, :, :], AF.Identity,
                        bias=bout_sb[:, m, None], scale=1.0,
                    )
                else:
                    nc.vector.tensor_scalar_add(dst, pp[:, :, :], bout_sb[:, m, None])
                if g == NG - 1:
                    nc.sync.dma_start(OUTS[j][m * P:(m + 1) * P, :, :], stage[:])

            # ---- GRU steps ----
            for t in range(STEPS):
                j, o = divmod(t, BS)
                ps = psg.tile([P, GM, BATCH], F32, tag="gates", name="ps")
                # preload step-invariant gate constants into all 12 groups
                nc.tensor.matmul(
                    ps[:, :, :], id_r, C_r, start=True, stop=False,
                    skip_group_check=True,
                )
                if t == 0:
                    prev = x0_sb
                else:
                    jp, op = divmod(t - 1, BS)
                    prev = resT[jp][:, :, :, op]
                # gate matmuls, r groups first, then n, then z; the Act ops
                # are emitted right after the matmuls they consume so the
                # tile framework places a sync point (event) there.
                for m in (0, 1, 2, 3):
                    for k in range(KO):
                        nc.tensor.matmul(
                            ps[:, m, :], wih[:, k, m, :], prev[:, k, :],
                            start=False, stop=(k == KO - 1),
                            skip_group_check=True,
                        )
                # r: tanh(0.5*(gi+gh+b)) ; r = (1+t_r)/2
                t_r = sp.tile([P, KO, BATCH], F32, tag="tr")
                nc.scalar.activation(t_r[:], ps[:, 0:4, :], AF.Tanh, scale=0.5)
                for m in (8, 9, 10, 11):
                    for k in range(KO):
                        nc.tensor.matmul(
                            ps[:, m, :], wih[:, k, m, :], prev[:, k, :],
                            start=False, stop=(k == KO - 1),
                            skip_group_check=True,
                        )
                for m in (4, 5, 6, 7):
                    for k in range(KO):
                        nc.tensor.matmul(
                            ps[:, m, :], wih[:, k, m, :], prev[:, k, :],
                            start=False, stop=(k == KO - 1),
                            skip_group_check=True,
                        )
                t_z = sp.tile([P, KO, BATCH], F32, tag="tz")
                nc.scalar.activation(t_z[:], ps[:, 4:8, :], AF.Tanh, scale=0.5)
                # interleave pending projection work into the step's PE slack
                for _ in range(UNITS_PER_STEP):
                    if proj_fifo:
                        emit_unit()
                # n = tanh(gi_n + b_ih_n + r*gh_n) with r*gh_n = hn2 + t_r*hn2
                a = sp.tile([P, KO, BATCH], F32, tag="a")
                nc.gpsimd.tensor_mul(a[:], t_r[:], hn2[:])
                sn = sp.tile([P, KO, BATCH], F32, tag="sn")
                nc.gpsimd.tensor_add(sn[:], ps[:, 8:12, :], a[:])
                n = sp.tile([P, KO, BATCH], F32, tag="n")
                nc.scalar.activation(n[:], sn[:], AF.Tanh, scale=1.0)
                # h = (1-z)*n + z*h0, via q' = 0.5*(h0-n):
                #   w2 = t_z*q' ; p2 = h0 - q' ; h = w2 + p2   (all Pool, b2b)
                q = sp.tile([P, KO, BATCH], F32, tag="q")
                nc.gpsimd.scalar_tensor_tensor(
                    q[:], n[:], -0.5, h0_half[:], ALU.mult, ALU.add
                )
                w2 = sp.tile([P, KO, BATCH], F32, tag="w2")
                nc.gpsimd.tensor_mul(w2[:], t_z[:], q[:])
                p2 = sp.tile([P, KO, BATCH], F32, tag="p2")
                nc.gpsimd.scalar_tensor_tensor(
                    p2[:], q[:], -1.0, h0T[:], ALU.mult, ALU.add
                )
                nc.gpsimd.tensor_add(resT[j][:, :, :, o], w2[:], p2[:])
                if o == BS - 1:
                    proj_fifo.extend(
                        (j, m, g) for m in range(MT) for g in range(NG)
                    )
            tail[0] = True
            while proj_fifo:
                emit_unit()

    nc.compile()
    return nc


def _shard_inputs(feat, W_hp, b_hp, W_ih, W_hh, b_ih, b_hh, embed, W_out, b_out):
    bf = ml_dtypes.bfloat16
    featT = np.ascontiguousarray(feat.T, dtype=np.float32)
    WhpT = np.ascontiguousarray(W_hp.T, dtype=np.float32)
    WihT = np.ascontiguousarray(W_ih.T).astype(bf)
    WhhT = np.ascontiguousarray(W_hh.T, dtype=np.float32)
    x0T = np.ascontiguousarray(
        np.repeat(np.asarray(embed)[SOS][:, None], BATCH, axis=1)
    ).astype(bf)
    ident = np.eye(P, dtype=np.float32)
    Wo = np.zeros((NCORES * VPAD, HID), np.float32)
    Wo[:VOCAB] = W_out
    bo = np.zeros((NCORES * VPAD,), np.float32)
    bo[:VOCAB] = b_out
    common = dict(
        featT=featT, WhpT=WhpT, WihT=WihT, WhhT=WhhT,
        b_ih=np.asarray(b_ih, np.float32), b_hh=np.asarray(b_hh, np.float32),
        b_hp=np.asarray(b_hp, np.float32), x0T=x0T, ident=ident,
    )
    in_maps = []
    for c in range(NCORES):
        sl = slice(c * VPAD, (c + 1) * VPAD)
        m = dict(common)
        m["WoutT"] = np.ascontiguousarray(Wo[sl].T).astype(bf)
        m["b_out"] = bo[sl].copy()
        in_maps.append(m)
    return in_maps


def kernel(**inputs):
    global LAST_RESULTS
    args = {k: np.asarray(v) for k, v in inputs.items()}
    in_maps = _shard_inputs(
        args["feat"], args["W_hp"], args["b_hp"], args["W_ih"], args["W_hh"],
        args["b_ih"], args["b_hh"], args["embed"], args["W_out"], args["b_out"],
    )
    nc = build()
    res = run_bass_kernel_spmd(nc, in_maps, core_ids=list(range(NCORES)))
    LAST_RESULTS = res
    per_core = []
    for r in res.results:
        blocks = [r[f"OUT{j}"] for j in range(NBLK)]   # each [VPAD, B, BS]
        per_core.append(np.concatenate(blocks, axis=2))  # [VPAD, B, T]
    full = np.concatenate(per_core, axis=0)              # [8*VPAD, B, T]
    out = full[:VOCAB].transpose(1, 0, 2)                # [B, V, T]
    return np.ascontiguousarray(out, dtype=np.float32)


# revision 15
# speedup vs baseline: 1.0125x; 1.0125x over previous
"""Trainium2 Bass kernel for the GRU caption model.

h0 = feat @ W_hp.T + b_hp; 200-step GRU whose hidden-proj gate pre-activations
are step-invariant; logits = h_t @ W_out.T + b_out -> [B, V, T].

v2 design (CoreSim cost-model driven):
- Vocab sharded 8 ways; every core runs the (latency-bound) GRU redundantly.
- GRU per-step chain minimized: step-invariant gate constants are preloaded
  into PSUM with a single N=384 fp32r identity matmul, the elementwise chain
  runs on Pool (GPSIMD, no SBUF/PSUM access-latency penalty) with the two
  tanhs on Act.  h_t is written once, in bf16, straight into the resT
  activation buffer that feeds both the next step's matmul and the
  projection.
- Projection in 4 t-blocks of 50 steps, bf16 weights/activations (1 cyc/row
  at any N), bias folded into the PSUM->SBUF copy (split between DVE+Pool).
- Output DMA: one transfer per (vocab-tile, t-block) with 6400-byte
  contiguous runs (full 360 GB/s); per-block DRAM tensors laid out
  [VPAD, B, BS], transposed/stitched on the host.
"""

import numpy as np
import ml_dtypes

import concourse.bass as bass
import concourse.mybir as mybir
import concourse.tile as tile
from concourse import bacc
from concourse.bass_utils import run_bass_kernel_spmd

F32 = mybir.dt.float32
F32R = mybir.dt.float32r
BF16 = mybir.dt.bfloat16
AF = mybir.ActivationFunctionType
ALU = mybir.AluOpType

VOCAB = 30522
HID = 512
FEAT = 2048
STEPS = 200
BATCH = 32
SOS = 101
NCORES = 8
P = 128
KO = HID // P          # 4 h-chunks
GM = 3 * HID // P      # 12 gate row-groups (r: 0-3, z: 4-7, n: 8-11)
KF = FEAT // P         # 16 feat chunks
VPAD = 3840            # per-core padded vocab rows = 30 * 128
MT = VPAD // P         # 30 vocab tiles per core
NBLK = 4
BS = STEPS // NBLK     # 50 steps per proj block
PBG = 4                # batch group size in proj (N = PBG*BS = 200)
UNITS_PER_STEP = 3     # proj (m,g) units interleaved per GRU step

LAST_RESULTS = None  # test harness introspection


def build():
    nc = bacc.Bacc("TRN2", target_bir_lowering=False, debug=False)

    featT = nc.dram_tensor("featT", [FEAT, BATCH], F32, kind="ExternalInput")
    WhpT = nc.dram_tensor("WhpT", [FEAT, HID], F32, kind="ExternalInput")
    WihT = nc.dram_tensor("WihT", [HID, 3 * HID], BF16, kind="ExternalInput")
    WhhT = nc.dram_tensor("WhhT", [HID, 3 * HID], F32, kind="ExternalInput")
    b_ih = nc.dram_tensor("b_ih", [3 * HID], F32, kind="ExternalInput")
    b_hh = nc.dram_tensor("b_hh", [3 * HID], F32, kind="ExternalInput")
    b_hp = nc.dram_tensor("b_hp", [HID], F32, kind="ExternalInput")
    x0T = nc.dram_tensor("x0T", [HID, BATCH], BF16, kind="ExternalInput")
    ident = nc.dram_tensor("ident", [P, P], F32, kind="ExternalInput")
    WoutT = nc.dram_tensor("WoutT", [HID, VPAD], BF16, kind="ExternalInput")
    b_out = nc.dram_tensor("b_out", [VPAD], F32, kind="ExternalInput")
    OUTS = [
        nc.dram_tensor(f"OUT{j}", [VPAD, BATCH, BS], F32, kind="ExternalOutput")
        for j in range(NBLK)
    ]

    with tile.TileContext(nc) as tc:
        with (
            tc.tile_pool(name="const", bufs=1) as const,
            tc.tile_pool(name="sp", bufs=4) as sp,
            tc.tile_pool(name="stg", bufs=3) as stg,
            tc.tile_pool(name="psg", bufs=2, space="PSUM") as psg,
            tc.tile_pool(name="psc", bufs=2, space="PSUM") as psc,
            tc.tile_pool(name="psp", bufs=4, space="PSUM") as psp,
        ):
            # ---- constants into SBUF (few, large DMAs) ----
            featT_sb = const.tile([P, KF, BATCH], F32, tag="featsb")
            nc.sync.dma_start(featT_sb[:], featT.rearrange("(k p) b -> p k b", p=P))
            whp_sb = const.tile([P, KF, HID], F32, tag="whp")
            nc.sync.dma_start(whp_sb[:], WhpT.rearrange("(k p) h -> p k h", p=P))
            bhp_sb = const.tile([P, KO], F32, tag="bhp")
            nc.sync.dma_start(bhp_sb[:], b_hp.rearrange("(m p) -> p m", p=P))
            bih_sb = const.tile([P, GM], F32, tag="bih")
            nc.sync.dma_start(bih_sb[:], b_ih.rearrange("(m p) -> p m", p=P))
            bhh_sb = const.tile([P, GM], F32, tag="bhh")
            nc.sync.dma_start(bhh_sb[:], b_hh.rearrange("(m p) -> p m", p=P))
            id_sb = const.tile([P, P], F32, tag="id")
            nc.sync.dma_start(id_sb[:], ident[:, :])
            x0_sb = const.tile([P, KO, BATCH], BF16, tag="x0")
            nc.sync.dma_start(x0_sb[:], x0T.rearrange("(k p) b -> p k b", p=P))
            whh_sb = const.tile([P, KO, 3 * HID], F32, tag="whh")
            nc.sync.dma_start(whh_sb[:], WhhT.rearrange("(k p) g -> p k g", p=P))
            wih = const.tile([P, KO, GM, P], BF16, tag="wih")
            nc.sync.dma_start(
                wih[:], WihT.rearrange("(k p) (m c) -> p k m c", p=P, c=P)
            )
            wout_sb = const.tile([P, KO, VPAD], BF16, tag="wout")
            nc.sync.dma_start(
                wout_sb[:], WoutT.rearrange("(k p) v -> p k v", p=P)
            )
            bout_sb = const.tile([P, MT], F32, tag="bout")
            nc.sync.dma_start(bout_sb[:], b_out.rearrange("(m p) -> p m", p=P))

            # ---- h0 = feat @ W_hp.T + b_hp (fp32, exact) ----
            ps_h = psg.tile([P, GM, BATCH], F32, tag="gates", name="psh")
            for ko in range(KO):
                for kf in range(KF):
                    nc.tensor.matmul(
                        ps_h[:, ko, :],
                        whp_sb[:, kf, ko * P:(ko + 1) * P],
                        featT_sb[:, kf, :],
                        start=(kf == 0), stop=(kf == KF - 1),
                    )
            h0T = const.tile([P, KO, BATCH], F32, tag="h0T")
            for ko in range(KO):
                nc.scalar.activation(
                    h0T[:, ko, :], ps_h[:, ko, :], AF.Identity,
                    bias=bhp_sb[:, ko, None], scale=1.0,
                )
            h0_half = const.tile([P, KO, BATCH], F32, tag="h0h")
            nc.scalar.mul(h0_half[:], h0T[:], 0.5)

            # ---- gh = h0 @ W_hh.T + b_hh (fp32, exact; step-invariant) ----
            ps_g = psg.tile([P, GM, BATCH], F32, tag="gates", name="psg2")
            for m in range(GM):
                for k in range(KO):
                    nc.tensor.matmul(
                        ps_g[:, m, :],
                        whh_sb[:, k, m * P:(m + 1) * P],
                        h0T[:, k, :],
                        start=(k == 0), stop=(k == KO - 1),
                    )
            ghT = const.tile([P, GM, BATCH], F32, tag="ghT")
            for m in range(GM):
                nc.scalar.activation(
                    ghT[:, m, :], ps_g[:, m, :], AF.Identity,
                    bias=bhh_sb[:, m, None], scale=1.0,
                )
            # hn2 = 0.5*gh_n (used every step by the r-gating of n)
            hn2 = const.tile([P, KO, BATCH], F32, tag="hn2")
            nc.scalar.mul(hn2[:], ghT[:, 8:12, :], 0.5)
            # C: per-step PSUM preload. rz: gh_rz + b_ih_rz ; n: hn2 + b_ih_n
            C = const.tile([P, GM, BATCH], F32, tag="C")
            nc.vector.tensor_add(
                C[:, 0:8, :], ghT[:, 0:8, :],
                bih_sb[:, 0:8, None].to_broadcast((P, 8, BATCH)),
            )
            nc.vector.tensor_add(
                C[:, 8:12, :], hn2[:],
                bih_sb[:, 8:12, None].to_broadcast((P, KO, BATCH)),
            )

            # resT blocks: h_t in bf16, feeds next-step matmul AND projection
            resT = []
            for j in range(NBLK):
                rt = const.tile(
                    [P, KO, BATCH, BS], BF16, tag=f"resT{j}", name=f"resT{j}"
                )
                resT.append(rt)

            id_r = id_sb.bitcast(F32R)
            C_r = C.bitcast(F32R)

            NG = BATCH // PBG  # 8 batch groups per vocab tile
            proj_fifo = []     # pending (j, m, g) units
            stage_cur = [None]
            tail = [False]

            def emit_unit():
                j, m, g = proj_fifo.pop(0)
                if g == 0:
                    stage_cur[0] = stg.tile(
                        [P, BATCH, BS], F32, tag="stage", name="stage"
                    )
                stage = stage_cur[0]
                pp = psp.tile([P, PBG, BS], F32, tag="pp", name="pp")
                for k in range(KO):
                    nc.tensor.matmul(
                        pp[:, :, :],
                        wout_sb[:, k, m * P:(m + 1) * P],
                        resT[j][:, k, PBG * g:PBG * g + PBG, :],
                        start=(k == 0), stop=(k == KO - 1),
                    )
                dst = stage[:, PBG * g:PBG * g + PBG, :]
                # During the GRU, copies live on DVE only (Pool/Act host the
                # latency-critical chain); in the tail all engines pitch in.
                if tail[0] and g % 3 == 0:
                    nc.gpsimd.tensor_scalar_add(dst, pp[:, :, :], bout_sb[:, m, None])
                elif tail[0] and g % 3 == 1:
                    nc.scalar.activation(
                        dst, pp[:, :, :], AF.Identity,
                        bias=bout_sb[:, m, None], scale=1.0,
                    )
                else:
                    nc.vector.tensor_scalar_add(dst, pp[:, :, :], bout_sb[:, m, None])
                if g == NG - 1:
                    nc.sync.dma_start(OUTS[j][m * P:(m + 1) * P, :, :], stage[:])

            # ---- GRU steps ----
            for t in range(STEPS):
                j, o = divmod(t, BS)
                ps = psg.tile([P, GM, BATCH], F32, tag="gates", name="ps")
                # preload step-invariant gate constants into all 12 groups
                nc.tensor.matmul(
                    ps[:, :, :], id_r, C_r, start=True, stop=False,
                    skip_group_check=True,
                )
                if t == 0:
                    prev = x0_sb
                else:
                    jp, op = divmod(t - 1, BS)
                    prev = resT[jp][:, :, :, op]
                # Emission order = dependency barriers: each op waits for all
                # previously-emitted ops on engines it reads from.  Order:
                # r-mms, t_r, z-mms, t_z, n-mms, a, sn, n, tail, proj units.
                scr = psc.tile([P, 16, BATCH], F32, tag="scr", name="scr")
                for m in (0, 1, 2, 3):
                    for k in range(KO):
                        nc.tensor.matmul(
                            ps[:, m, :], wih[:, k, m, :], prev[:, k, :],
                            start=False, stop=(k == KO - 1),
                            skip_group_check=True,
                        )
                # r: tanh(0.5*(gi+gh+b)) ; r = (1+t_r)/2   (PSUM -> PSUM)
                t_r = scr[:, 0:4, :]
                nc.scalar.activation(t_r, ps[:, 0:4, :], AF.Tanh, scale=0.5)
                for m in (4, 5, 6, 7):
                    for k in range(KO):
                        nc.tensor.matmul(
                            ps[:, m, :], wih[:, k, m, :], prev[:, k, :],
                            start=False, stop=(k == KO - 1),
                            skip_group_check=True,
                        )
                t_z = scr[:, 4:8, :]
                nc.scalar.activation(t_z, ps[:, 4:8, :], AF.Tanh, scale=0.5)
                for m in (8, 9, 10, 11):
                    for k in range(KO):
                        nc.tensor.matmul(
                            ps[:, m, :], wih[:, k, m, :], prev[:, k, :],
                            start=False, stop=(k == KO - 1),
                            skip_group_check=True,
                        )
                # n = tanh(gi_n + b_ih_n + r*gh_n) with r*gh_n = hn2 + t_r*hn2
                a = sp.tile([P, KO, BATCH], F32, tag="a")
                nc.gpsimd.tensor_mul(a[:], t_r, hn2[:])
                sn = scr[:, 8:12, :]
                nc.gpsimd.tensor_add(sn, ps[:, 8:12, :], a[:])
                n = scr[:, 12:16, :]
                nc.scalar.activation(n, sn, AF.Tanh, scale=1.0)
                # h = (1-z)*n + z*h0, via q' = 0.5*(h0-n):
                #   w2 = t_z*q' ; p2 = h0 - q' ; h = w2 + p2   (all Pool, b2b)
                q = sp.tile([P, KO, BATCH], F32, tag="q")
                nc.gpsimd.scalar_tensor_tensor(
                    q[:], n, -0.5, h0_half[:], ALU.mult, ALU.add
                )
                w2 = sp.tile([P, KO, BATCH], F32, tag="w2")
                nc.gpsimd.tensor_mul(w2[:], t_z, q[:])
                p2 = sp.tile([P, KO, BATCH], F32, tag="p2")
                nc.gpsimd.scalar_tensor_tensor(
                    p2[:], q[:], -1.0, h0T[:], ALU.mult, ALU.add
                )
                nc.gpsimd.tensor_add(resT[j][:, :, :, o], w2[:], p2[:])
                # interleave pending projection work into the step's PE slack
                for _ in range(UNITS_PER_STEP):
                    if proj_fifo:
                        emit_unit()
                if o == BS - 1:
                    proj_fifo.extend(
                        (j, m, g) for m in range(MT) for g in range(NG)
                    )
            tail[0] = True
            while proj_fifo:
                emit_unit()

    nc.compile()
    return nc


def _shard_inputs(feat, W_hp, b_hp, W_ih, W_hh, b_ih, b_hh, embed, W_out, b_out):
    bf = ml_dtypes.bfloat16
    featT = np.ascontiguousarray(feat.T, dtype=np.float32)
    WhpT = np.ascontiguousarray(W_hp.T, dtype=np.float32)
    WihT = np.ascontiguousarray(W_ih.T).astype(bf)
    WhhT = np.ascontiguousarray(W_hh.T, dtype=np.float32)
    x0T = np.ascontiguousarray(
        np.repeat(np.asarray(embed)[SOS][:, None], BATCH, axis=1)
    ).astype(bf)
    ident = np.eye(P, dtype=np.float32)
    Wo = np.zeros((NCORES * VPAD, HID), np.float32)
    Wo[:VOCAB] = W_out
    bo = np.zeros((NCORES * VPAD,), np.float32)
    bo[:VOCAB] = b_out
    common = dict(
        featT=featT, WhpT=WhpT, WihT=WihT, WhhT=WhhT,
        b_ih=np.asarray(b_ih, np.float32), b_hh=np.asarray(b_hh, np.float32),
        b_hp=np.asarray(b_hp, np.float32), x0T=x0T, ident=ident,
    )
    in_maps = []
    for c in range(NCORES):
        sl = slice(c * VPAD, (c + 1) * VPAD)
        m = dict(common)
        m["WoutT"] = np.ascontiguousarray(Wo[sl].T).astype(bf)
        m["b_out"] = bo[sl].copy()
        in_maps.append(m)
    return in_maps


def kernel(**inputs):
    global LAST_RESULTS
    args = {k: np.asarray(v) for k, v in inputs.items()}
    in_maps = _shard_inputs(
        args["feat"], args["W_hp"], args["b_hp"], args["W_ih"], args["W_hh"],
        args["b_ih"], args["b_hh"], args["embed"], args["W_out"], args["b_out"],
    )
    nc = build()
    res = run_bass_kernel_spmd(nc, in_maps, core_ids=list(range(NCORES)))
    LAST_RESULTS = res
    per_core = []
    for r in res.results:
        blocks = [r[f"OUT{j}"] for j in range(NBLK)]   # each [VPAD, B, BS]
        per_core.append(np.concatenate(blocks, axis=2))  # [VPAD, B, T]
    full = np.concatenate(per_core, axis=0)              # [8*VPAD, B, T]
    out = full[:VOCAB].transpose(1, 0, 2)                # [B, V, T]
    return np.ascontiguousarray(out, dtype=np.float32)
